# revision 52
# baseline (speedup 1.0000x reference)
"""Multi-head self-attention on 8 TRN2 NeuronCores.

Problem: x(4,2048,1024), Wq(8,1024,128), Wk/Wv(1024,128), Wo(1024,1024) fp32.
out = softmax(Q K^T / sqrt(128)) V -> concat heads -> @ Wo.

Sharding: (batch, query-half) across 8 cores — core c handles batch c//2,
query rows [(c%2)*1024, (c%2)*1024+1024). K/V cover the full sequence of the
batch, so each core computes them locally from its x slice; no collectives.

Numerics: scores have std ~1024 and softmax is near-one-hot, so the
x->Q/K->scores chain needs ~fp32 precision. bf16 matmuls with hi/lo split
operands ("split3": Ah*Bh + Ah*Bl + Al*Bh, fp32 PSUM accumulation) give
~5e-6 relative matmul error at 3 cycles/row (native fp32 is 4). The x and
weight splits are precomputed on the host. V/ctx/Wo paths are plain bf16.

Layouts (partition dim first):
  xT (E,S) host-transposed; K^T (O,S) = sum_e Wk[e].T-stationary @ xT[e];
  Q_h^T (O,Sq) likewise (Wq pre-scaled by 1/sqrt(O) on host);
  scores tile (128q, 2048s) = Q^T-slice-stationary @ K^T-moving, fp32 PSUM,
  bank-chunk-major so each 512-col bank finishes early;
  softmax per q-row: per-bank DVE reduce_max -> combine(negate) -> per-bank
  ACT exp(bias=-max, accum_out=den chunk) -> den sum -> 1/den -> DVE scale;
  P transposed 128x128 via PE right after each q-tile (PE gap filler);
  ctx^T (O,Sq) = V-stationary @ P^T-moving; out (Sq,E) = ctx-slices-stationary
  @ Wo-moving (natural output layout).
"""
import numpy as np
import ml_dtypes

B, S, E, H, O = 4, 2048, 1024, 8, 128
SQ = S // 2          # query rows per core
NCORES = 8
ET = E // 128        # 8 e-tiles
ST = S // 128        # 16 s-tiles
QT = SQ // 128       # 8 q-tiles
NB = S // 512        # 4 score banks per q-tile
EC = E // 512        # 2 out-proj column chunks

_compiled = None     # cache so repeated kernel() calls skip rebuild


def _build():
    import concourse.bass as bass
    import concourse.mybir as mybir
    import concourse.tile as tile
    from concourse import bacc
    from concourse.masks import make_identity

    F32 = mybir.dt.float32
    BF16 = mybir.dt.bfloat16
    PS = bass.MemorySpace.PSUM
    EXP = mybir.ActivationFunctionType.Exp

    nc = bacc.Bacc("TRN2", target_bir_lowering=False, debug=False,
                   enable_asserts=True)

    # xkv columns are pre-permuted per core so its query half is always
    # columns [0, SQ) — attention is permutation-invariant over the key axis,
    # so the same NEFF slices queries identically on every core.
    d_xkvh = nc.dram_tensor("xkvh", (E, S), BF16, kind="ExternalInput").ap()
    d_xkvl = nc.dram_tensor("xkvl", (E, S), BF16, kind="ExternalInput").ap()
    d_wqh = nc.dram_tensor("wqh", (H, E, O), BF16, kind="ExternalInput").ap()
    d_wql = nc.dram_tensor("wql", (H, E, O), BF16, kind="ExternalInput").ap()
    d_wkh = nc.dram_tensor("wkh", (E, O), BF16, kind="ExternalInput").ap()
    d_wkl = nc.dram_tensor("wkl", (E, O), BF16, kind="ExternalInput").ap()
    d_wvh = nc.dram_tensor("wvh", (E, O), BF16, kind="ExternalInput").ap()
    d_woh = nc.dram_tensor("woh", (H * O, E), BF16, kind="ExternalInput").ap()
    d_out = nc.dram_tensor("out", (SQ, E), F32, kind="ExternalOutput").ap()

    with tile.TileContext(nc) as tc:
        with (
            tc.tile_pool(name="persist", bufs=1) as persist,
            tc.tile_pool(name="tiny", bufs=24) as tiny,
        ):
            ident = persist.tile([128, 128], BF16, tag="ident")

            # DMA queue = issuing engine; spread big loads across four queues
            # (sync/scalar are HWDGE, vector/gpsimd SWDGE) — a single queue
            # streams only ~38 GB/s
            dmae = [nc.sync, nc.scalar, nc.gpsimd]

            wo_sb = persist.tile([128, H, E], BF16, tag="wo")

            kth = persist.tile([128, S], BF16, tag="kth")
            ktl = persist.tile([128, S], BF16, tag="ktl")
            qth = persist.tile([128, H, SQ], BF16, tag="qth")
            qtl = persist.tile([128, H, SQ], BF16, tag="qtl")
            v_sb = persist.tile([128, ST, O], BF16, tag="v")

            # ---------------- prologue: K^T, V, Q^T projections ----------
            with tc.tile_pool(name="xp", bufs=1) as xp:
                wkh = xp.tile([128, ET, O], BF16, tag="wkh")
                wkl = xp.tile([128, ET, O], BF16, tag="wkl")
                nc.sync.dma_start(wkh[:], d_wkh.rearrange("(t p) o -> p t o", p=128))
                nc.scalar.dma_start(wkl[:], d_wkl.rearrange("(t p) o -> p t o", p=128))
                xkvh = xp.tile([128, ET, S], BF16, tag="xkvh")
                xkvl = xp.tile([128, ET, S], BF16, tag="xkvl")
                wqh = xp.tile([128, H, ET, O], BF16, tag="wqh")
                wql = xp.tile([128, H, ET, O], BF16, tag="wql")
                wvh = xp.tile([128, ET, O], BF16, tag="wvh")
                # Q-first streaming on the fast gpsimd SWDGE queue: head-0
                # weights, then the query-half columns (all Q needs), then the
                # rest trickles in under the Q phase's ~80us of PE work
                nc.gpsimd.dma_start(
                    wqh[:, 0, :, :], d_wqh[0].rearrange("(t p) o -> p t o", p=128))
                nc.gpsimd.dma_start(
                    wql[:, 0, :, :], d_wql[0].rearrange("(t p) o -> p t o", p=128))
                for e in range(ET):
                    nc.gpsimd.dma_start(
                        xkvh[:, e, 0:SQ], d_xkvh[e * 128:(e + 1) * 128, 0:SQ])
                    nc.gpsimd.dma_start(
                        xkvl[:, e, 0:SQ], d_xkvl[e * 128:(e + 1) * 128, 0:SQ])
                for h in range(1, H):
                    nc.gpsimd.dma_start(
                        wqh[:, h, :, :],
                        d_wqh[h].rearrange("(t p) o -> p t o", p=128))
                    nc.gpsimd.dma_start(
                        wql[:, h, :, :],
                        d_wql[h].rearrange("(t p) o -> p t o", p=128))
                for e in range(ET):
                    nc.gpsimd.dma_start(
                        xkvh[:, e, SQ:S], d_xkvh[e * 128:(e + 1) * 128, SQ:S])
                    nc.gpsimd.dma_start(
                        xkvl[:, e, SQ:S], d_xkvl[e * 128:(e + 1) * 128, SQ:S])
                nc.scalar.dma_start(wvh[:], d_wvh.rearrange("(t p) o -> p t o", p=128))
                nc.scalar.dma_start(
                    wo_sb[:], d_woh.rearrange("(h p) e -> p h e", p=128))
                make_identity(nc, ident[:])

                # Q^T per head first (needs only the query-half columns)
                with tc.tile_pool(name="qp", bufs=2, space=PS) as qp:
                    for h in range(H):
                        q_ps = qp.tile([128, SQ], F32, tag="qtps")
                        for e in range(ET):
                            for ti, (w, xx) in enumerate(
                                ((wqh, xkvh), (wqh, xkvl), (wql, xkvh))
                            ):
                                for c in range(SQ // 512):
                                    nc.tensor.matmul(
                                        q_ps[:, c * 512:(c + 1) * 512],
                                        w[:, h, e, :],
                                        xx[:, e, c * 512:(c + 1) * 512],
                                        start=(e == 0 and ti == 0),
                                        stop=(e == ET - 1 and ti == 2),
                                    )
                        nc.scalar.copy(qth[:, h, :], q_ps[:])
                        nc.vector.tensor_sub(qtl[:, h, :], q_ps[:], qth[:, h, :])

                # K^T and V^T share one PSUM scope (4+4 banks) so their
                # matmuls interleave and neither phase-transition stalls PE
                with (
                    tc.tile_pool(name="ktp", bufs=1, space=PS) as ktp,
                    tc.tile_pool(name="vtp", bufs=1, space=PS) as vtp,
                ):
                    kt_ps = ktp.tile([128, S], F32, tag="kt")
                    vt_ps = vtp.tile([128, S], F32, tag="vt")
                    for e in range(ET):
                        for ti, (w, xx) in enumerate(
                            ((wkh, xkvh), (wkh, xkvl), (wkl, xkvh))
                        ):
                            for c in range(NB):
                                nc.tensor.matmul(
                                    kt_ps[:, c * 512:(c + 1) * 512],
                                    w[:, e, :],
                                    xx[:, e, c * 512:(c + 1) * 512],
                                    start=(e == 0 and ti == 0),
                                    stop=(e == ET - 1 and ti == 2),
                                )
                        # V^T (o-part) with Wv stationary: 8 weight loads
                        for c in range(NB):
                            nc.tensor.matmul(
                                vt_ps[:, c * 512:(c + 1) * 512],
                                wvh[:, e, :],
                                xkvh[:, e, c * 512:(c + 1) * 512],
                                start=(e == 0),
                                stop=(e == ET - 1),
                            )
                    nc.scalar.copy(kth[:], kt_ps[:])
                    nc.vector.tensor_sub(ktl[:], kt_ps[:], kth[:])
                    vt_sb = xp.tile([128, S], BF16, tag="vtsb")
                    nc.scalar.copy(vt_sb[:], vt_ps[:])
                with tc.tile_pool(name="vsp", bufs=2, space=PS) as vsp:
                    for g in range(2):
                        v_st = vsp.tile([128, 8, 128], BF16, tag="vst")
                        for k in range(8):
                            st = g * 8 + k
                            nc.tensor.transpose(
                                v_st[:, k, :],
                                vt_sb[:, st * 128:(st + 1) * 128],
                                ident[:],
                            )
                        nc.vector.tensor_copy(
                            v_sb[:, g * 8:(g + 1) * 8, :], v_st[:])

            # ---------------- main: per-head attention ------------------
            # PSUM budget (8 banks): "acc1024" 2-bank tiles x3 bufs shared by
            # score-halves, ctx and out accumulators (6 banks) + one 2-bank
            # transpose staging tile.  Score halves cycle through 3 slots so
            # the next q-tile's matmuls never wait on this one's softmax.
            with (
                tc.tile_pool(name="p_pool", bufs=4) as p_pool,
                tc.tile_pool(name="pt_pool", bufs=2) as pt_pool,
                tc.tile_pool(name="ctx_pool", bufs=H) as ctx_pool,
                tc.tile_pool(name="acc_ps", bufs=3, space=PS) as acc_psp,
                tc.tile_pool(name="pt_ps", bufs=1, space=PS) as pt_psp,
                tc.tile_pool(name="o_sb", bufs=2) as o_sbp,
            ):
                HS = S // 2  # 1024-wide score half

                def emit_transposes(pt_h, p_qt, qt):
                    # runs one q-tile BEHIND the softmax pipeline: all deps
                    # are long resolved, so these are always-ready PE filler
                    # and the ACT copy never stalls the exp stream
                    pt_ps = pt_psp.tile([128, ST, 128], BF16, tag="ptps")
                    for st in range(ST):
                        nc.tensor.transpose(
                            pt_ps[:, st, :],
                            p_qt[:, st * 128:(st + 1) * 128],
                            ident[:],
                        )
                    cp = nc.scalar.copy if qt % 2 else nc.vector.tensor_copy
                    cp(pt_h[:, :, qt * 128:(qt + 1) * 128], pt_ps[:])

                MIN = mybir.AluOpType.min
                SUB = mybir.AluOpType.subtract
                ctxs = []

                def emit_ctx_half(state, qc):
                    # ctx^T (o-part, q-free) accumulated over s-tiles; lagged
                    # into the next head's score phase as PE filler, one
                    # 512-wide half-burst at a time to limit the disruption
                    pt_h = state["pt"]
                    ctx_h = state["ctx"]
                    if state["ct"] is None:
                        ct_ps = acc_psp.tile([128, SQ], F32, tag="acc1024")
                        state["ct"] = ct_ps
                    ct_ps = state["ct"]
                    for st in range(ST):
                        nc.tensor.matmul(
                            ct_ps[:, qc * 512:(qc + 1) * 512],
                            v_sb[:, st, :],
                            pt_h[:, st, qc * 512:(qc + 1) * 512],
                            start=(st == 0),
                            stop=(st == ST - 1),
                        )
                    nc.scalar.copy(
                        ctx_h[:, qc * 512:(qc + 1) * 512],
                        ct_ps[:, qc * 512:(qc + 1) * 512])

                pending_ctx = None
                for h in range(H):
                    pt_h = pt_pool.tile([128, ST, SQ], BF16, tag="pt")
                    lagged = None
                    for qt in range(QT):
                        # flash-style: each half gets a LOCAL max + exp so its
                        # PSUM slot frees without waiting for the other half;
                        # tiny per-partition factors fix up the normalization.
                        nm2 = tiny.tile([128, 2], F32, tag="nm2")
                        den2 = tiny.tile([128, 2], F32, tag="den2")
                        p_qt = p_pool.tile([128, S], BF16, tag="p")
                        for sh in range(2):
                            s_ps = acc_psp.tile([128, HS], F32, tag="acc1024")
                            for ti, (qq, kk) in enumerate(
                                ((qth, kth), (qth, ktl), (qtl, kth))
                            ):
                                for c in range(2):
                                    nc.tensor.matmul(
                                        s_ps[:, c * 512:(c + 1) * 512],
                                        qq[:, h, qt * 128:(qt + 1) * 128],
                                        kk[:, sh * HS + c * 512:
                                           sh * HS + (c + 1) * 512],
                                        start=(ti == 0),
                                        stop=(ti == 2),
                                    )
                            nc.vector.reduce_max(
                                out=nm2[:, sh:sh + 1], in_=s_ps[:],
                                axis=mybir.AxisListType.X, negate=True,
                            )
                            nc.scalar.activation(
                                p_qt[:, sh * HS:(sh + 1) * HS],
                                s_ps[:],
                                EXP, bias=nm2[:, sh:sh + 1], scale=1.0,
                                accum_out=den2[:, sh:sh + 1],
                            )
                        # fixup: p *= exp(m_sh - m_glob) / den_glob, all [128,·]
                        nmg = tiny.tile([128, 1], F32, tag="nmg")
                        nc.vector.tensor_reduce(
                            out=nmg[:], in_=nm2[:],
                            axis=mybir.AxisListType.X, op=MIN,
                        )
                        f2 = tiny.tile([128, 2], F32, tag="f2")
                        nc.scalar.activation(
                            f2[:], nm2[:], EXP, bias=nmg[:], scale=-1.0)
                        t2 = tiny.tile([128, 2], F32, tag="t2")
                        nc.vector.tensor_mul(t2[:], den2[:], f2[:])
                        den = tiny.tile([128, 1], F32, tag="den")
                        nc.vector.tensor_add(den[:], t2[:, 0:1], t2[:, 1:2])
                        invden = tiny.tile([128, 1], F32, tag="invden")
                        nc.vector.reciprocal(invden[:], den[:])
                        for sh in range(2):
                            nc.vector.tensor_scalar(
                                out=p_qt[:, sh * HS:(sh + 1) * HS],
                                in0=p_qt[:, sh * HS:(sh + 1) * HS],
                                scalar1=f2[:, sh:sh + 1],
                                scalar2=invden[:],
                                op0=mybir.AluOpType.mult,
                                op1=mybir.AluOpType.mult,
                            )

                        if lagged is not None:
                            emit_transposes(pt_h, *lagged)
                        lagged = (p_qt, qt)
                        if pending_ctx is not None and qt in (1, 3):
                            emit_ctx_half(pending_ctx, qt // 2)
                            if qt == 3:
                                pending_ctx = None
                    emit_transposes(pt_h, *lagged)
                    ctx_h = ctx_pool.tile([128, SQ], BF16, tag="ctx")
                    pending_ctx = {"pt": pt_h, "ct": None, "ctx": ctx_h}
                    ctxs.append(ctx_h)
                for qc in range(2):
                    emit_ctx_half(pending_ctx, qc)

                # ------- out (q-part, e-free) = sum_h ctx_h^T-slices @ Wo_h
                for qt in range(QT):
                    o_ps = acc_psp.tile([128, E], F32, tag="acc1024")
                    for h in range(H):
                        for ec in range(EC):
                            nc.tensor.matmul(
                                o_ps[:, ec * 512:(ec + 1) * 512],
                                ctxs[h][:, qt * 128:(qt + 1) * 128],
                                wo_sb[:, h, ec * 512:(ec + 1) * 512],
                                start=(h == 0),
                                stop=(h == H - 1),
                            )
                    o_sb = o_sbp.tile([128, E], F32, tag="osb")
                    nc.scalar.copy(o_sb[:], o_ps[:])
                    nc.gpsimd.dma_start(
                        d_out[qt * 128:(qt + 1) * 128, :], o_sb[:])

    nc.compile()
    return nc


def _split(a):
    """fp32 -> (hi, lo) bf16 pair with hi + lo ~= a."""
    hi = a.astype(ml_dtypes.bfloat16)
    lo = (a - hi.astype(np.float32)).astype(ml_dtypes.bfloat16)
    return hi, lo


def kernel(x, Wq, Wk, Wv, Wo):
    global _compiled
    from concourse.bass_utils import run_bass_kernel_spmd

    x = np.asarray(x, dtype=np.float32)
    Wq = np.asarray(Wq, dtype=np.float32)
    Wk = np.asarray(Wk, dtype=np.float32)
    Wv = np.asarray(Wv, dtype=np.float32)
    Wo = np.asarray(Wo, dtype=np.float32)

    if _compiled is None:
        _compiled = _build()
    nc = _compiled

    scale = np.float32(1.0 / np.sqrt(O))
    wqh, wql = _split(Wq.astype(np.float32) * scale)
    wkh, wkl = _split(Wk.astype(np.float32))
    wvh = Wv.astype(ml_dtypes.bfloat16)
    woh = Wo.astype(ml_dtypes.bfloat16)

    in_maps = []
    xsplits = {}
    for b in range(B):
        xsplits[b] = _split(np.ascontiguousarray(x[b].T))  # (E, S) fp32
    for c in range(NCORES):
        b, half = divmod(c, 2)
        xh, xl = xsplits[b]
        if half == 0:
            ph, pl = xh, xl
        else:
            # rotate so this core's query half occupies columns [0, SQ);
            # attention is permutation-invariant over the key/value axis
            ph = np.ascontiguousarray(np.roll(xh, SQ, axis=1))
            pl = np.ascontiguousarray(np.roll(xl, SQ, axis=1))
        in_maps.append({
            "xkvh": ph, "xkvl": pl,
            "wqh": wqh, "wql": wql,
            "wkh": wkh, "wkl": wkl, "wvh": wvh, "woh": woh,
        })

    res = run_bass_kernel_spmd(nc, in_maps, core_ids=list(range(NCORES)))

    out = np.empty((B, S, E), dtype=np.float32)
    for c in range(NCORES):
        b, half = divmod(c, 2)
        out[b, half * SQ:(half + 1) * SQ, :] = res.results[c]["out"]
    return out
